# revision 1
# baseline (speedup 1.0000x reference)
"""Batched EKF negative-log-likelihood loss on 8 Trainium2 NeuronCores.

Data-parallel Bass/Tile kernel: the N=8192 segments are sharded 1024 per
core, laid out as 128 partitions x 8 segments.  The EKF state is kept in
permuted + DT-scaled coordinates [x, y, th, DT*vx, DT*vy, DT*om] so the
transition Jacobian becomes F = [[I, I], [0, D]] with D = diag(d0, d1, a55)
- making F P F^T four strided elementwise ops.  The 3x3 innovation
covariance is inverted via a replicated 5x5 layout that yields the full
cofactor matrix in three vector ops.  tanh is computed via exp so that exp
and ln share one ScalarE activation-table set (no per-step table swaps).
"""

import os
import numpy as np

DT = 1.0 / 120.0
G = 9.81
K_SIGN = 100.0
TWO_PI = 2.0 * np.pi

N_CORES = 8
N_SEG = 8192
T_STEPS = 64
SEG_PER_CORE = N_SEG // N_CORES      # 1024
SLOTS = SEG_PER_CORE // 128          # 8 segments per partition

# ---------------------------------------------------------------------------
# Bass kernel builder
# ---------------------------------------------------------------------------


def _apx(tile_handle, off, *dims):
    """AP into `tile_handle` at element offset `off` with explicit
    (step, count) free dims, all 128 partitions."""
    from concourse.ap import AP

    base = tile_handle[:]
    ap = [list(base.ap[0])] + [[s, c] for (s, c) in dims]
    return AP(tensor=base.tensor, offset=base.offset + off, ap=ap)




def _eng(nc):
    """Offload engine for slack elementwise ops (env-switchable)."""
    import os as _os
    return nc.gpsimd if _os.environ.get("EKF_USE_POOL") else nc.vector

def _build_ekf(tc, out_ap, meas_ap, x0_ap, cst_ap, sc, T, reps=1):
    import concourse.bacc as bacc
    import concourse.mybir as mybir

    _patch_act_tables(bacc, mybir)
    nc = tc.nc
    f32 = mybir.dt.float32
    A = mybir.AluOpType
    AF = mybir.ActivationFunctionType
    S = SLOTS  # 8

    with tc.tile_pool(name="persist", bufs=1) as pp, \
         tc.tile_pool(name="temps", bufs=2) as tp:
        MEAS = pp.tile([128, S * 3 * T], f32, tag="MEAS")
        X = pp.tile([128, S * 6], f32, tag="X")
        P = pp.tile([128, S * 36], f32, tag="P")
        T3 = pp.tile([128, S * 3], f32, tag="T3")
        CST = pp.tile([128, S * 12], f32, tag="CST")
        MPACC = pp.tile([128, 9 * S * T], f32, tag="MPACC")
        LDACC = pp.tile([128, T], f32, tag="LDACC")

        if meas_ap.shape[1] == S * 3 * T:
            nc.sync.dma_start(MEAS[:], meas_ap)
        else:  # timing variants with T < T_STEPS: strided slice of dram
            from concourse.ap import AP as _AP
            src = _AP(tensor=meas_ap.tensor, offset=meas_ap.offset,
                      ap=[list(meas_ap.ap[0]),
                          [meas_ap.shape[1] // S, S], [1, 3 * T]])
            nc.sync.dma_start(_apx(MEAS, 0, (3 * T, S), (1, 3 * T)), src)
        nc.sync.dma_start(CST[:], cst_ap)
        # T3 slot 2 stays 1.0 forever (makes d slot2 come out as a55)
        nc.vector.memset(_apx(T3, 2, (3, S), (1, 1)), 1.0)

        # reps>1 re-runs the whole filter (timing variants only)
        for _rep in range(reps):
            nc.sync.dma_start(X[:], x0_ap)
            # P0 = diag(.01, .01, .01, .01*DT^2 x3) per segment
            nc.vector.memset(P[:], 0.0)
            nc.vector.memset(_apx(P, 0, (36, S), (7, 3)), 0.01)
            nc.vector.memset(_apx(P, 21, (36, S), (7, 3)), sc["p0v"])
            _ekf_steps(nc, tc, tp, sc, T, S, mybir, A, AF,
                       MEAS, X, P, T3, CST, MPACC, LDACC)

        # ---- final: per-partition sum of maha + logdet ----
        MT = tp.tile([128, 1], f32, tag="MT")
        LT = tp.tile([128, 1], f32, tag="LT")
        TOT = tp.tile([128, 1], f32, tag="TOT")
        nc.vector.tensor_reduce(MT[:], MPACC[:], mybir.AxisListType.X, A.add)
        nc.vector.tensor_reduce(LT[:], LDACC[:], mybir.AxisListType.X, A.add)
        nc.vector.tensor_tensor(TOT[:], MT[:], LT[:], A.add)
        nc.sync.dma_start(out_ap, TOT[:])


def _ekf_steps(nc, tc, tp, sc, T, S, mybir, A, AF,
               MEAS, X, P, T3, CST, MPACC, LDACC):
    """Per-step ops, emission-ordered for the in-order engine queues:
    DVE leads with work independent of the ScalarE exp/tanh chain, the
    S-matrix 5x5 replica is built with depth-1 copies spread over
    ScalarE/GPSIMD, and the loss side-chain rides on GPSIMD."""
    f32 = mybir.dt.float32
    if True:  # preserve indentation of the step loop
        for t in range(T):
            E = tp.tile([128, S * 2], f32, tag="E")
            EP = tp.tile([128, S * 2], f32, tag="EP")
            R2 = tp.tile([128, S * 2], f32, tag="R2")
            TSQ = tp.tile([128, S * 3], f32, tag="TSQ")
            D3 = tp.tile([128, S * 3], f32, tag="D3")
            Y = tp.tile([128, S * 3], f32, tag="Y")
            SEXT = tp.tile([128, S * 25], f32, tag="SEXT")
            PR1 = tp.tile([128, S * 9], f32, tag="PR1")
            PR2 = tp.tile([128, S * 9], f32, tag="PR2")
            C = tp.tile([128, S * 9], f32, tag="C")
            DP = tp.tile([128, S * 3], f32, tag="DP")
            DET = tp.tile([128, S], f32, tag="DET")
            REC = tp.tile([128, S], f32, tag="REC")
            LD = tp.tile([128, S], f32, tag="LD")
            SINV = tp.tile([128, S * 9], f32, tag="SINV")
            YY = tp.tile([128, S * 9], f32, tag="YY")
            KPR = tp.tile([128, S * 54], f32, tag="KPR")
            K6 = tp.tile([128, S * 18], f32, tag="K6")
            XP = tp.tile([128, S * 18], f32, tag="XP")
            XU = tp.tile([128, S * 6], f32, tag="XU")
            TMP0 = tp.tile([128, S * 36], f32, tag="TMP0")
            TMP1 = tp.tile([128, S * 36], f32, tag="TMP1")
            TMP2 = tp.tile([128, S * 36], f32, tag="TMP2")

            # [ACT] tanh chain start: E = exp(2K*vel), EP = E+1  (old vel)
            nc.scalar.activation(
                _apx(E, 0, (2, S), (1, 2)), _apx(X, 3, (6, S), (1, 2)),
                AF.Exp, scale=sc["kdt2"])
            nc.scalar.activation(
                _apx(EP, 0, (2, S), (1, 2)), _apx(E, 0, (2, S), (1, 2)),
                AF.Copy, bias=1.0)

            # [DVE] F P F^T row-add: independent of the tanh chain
            nc.vector.tensor_tensor(  # top rows += bottom rows
                _apx(P, 0, (36, S), (1, 18)), _apx(P, 0, (36, S), (1, 18)),
                _apx(P, 18, (36, S), (1, 18)), A.add)

            # [DVE] pos/th += vel'/om'  (old velocities)
            nc.vector.tensor_tensor(
                _apx(X, 0, (6, S), (1, 3)), _apx(X, 0, (6, S), (1, 3)),
                _apx(X, 3, (6, S), (1, 3)), A.add)

            # [DVE] innovation + wrap (needs pos only)
            nc.vector.tensor_tensor(
                _apx(Y, 0, (3, S), (1, 3)), _apx(MEAS, 3 * t, (3 * T, S), (1, 3)),
                _apx(X, 0, (6, S), (1, 3)), A.subtract)
            nc.vector.add_range_wrap(
                _apx(Y, 2, (3, S), (1, 1)), _apx(Y, 2, (3, S), (1, 1)),
                0.0, 1.5 * np.pi, TWO_PI)

            # [DVE] R2 = 1/(exp+1);  [ACT] t = 1-2*R2, t^2
            nc.vector.reciprocal(
                _apx(R2, 0, (2, S), (1, 2)), _apx(EP, 0, (2, S), (1, 2)))
            nc.scalar.activation(
                _apx(T3, 0, (3, S), (1, 2)), _apx(R2, 0, (2, S), (1, 2)),
                AF.Copy, bias=1.0, scale=-2.0)
            nc.scalar.activation(
                _apx(TSQ, 0, (3, S), (1, 3)), _apx(T3, 0, (3, S), (1, 3)),
                AF.Square)

            # [DVE] vel' *= a55 (incl om'), d = cK2*t^2 + dcon
            nc.scalar.activation(
                _apx(X, 3, (6, S), (1, 3)), _apx(X, 3, (6, S), (1, 3)),
                AF.Copy, scale=sc["a55"])
            nc.scalar.activation(
                _apx(D3, 0, (3, S), (1, 3)), _apx(TSQ, 0, (3, S), (1, 3)),
                AF.Copy, bias=sc["dcon"], scale=sc["cK2"])

            # [DVE] rest of P_pred = F P F^T + Q
            nc.vector.tensor_tensor(  # bottom rows *= d (broadcast over cols)
                _apx(P, 18, (36, S), (6, 3), (1, 6)),
                _apx(P, 18, (36, S), (6, 3), (1, 6)),
                _apx(D3, 0, (3, S), (1, 3), (0, 6)), A.mult)
            nc.vector.tensor_tensor(  # left cols += right cols
                _apx(P, 0, (36, S), (6, 6), (1, 3)),
                _apx(P, 0, (36, S), (6, 6), (1, 3)),
                _apx(P, 3, (36, S), (6, 6), (1, 3)), A.add)

            # ---- SEXT[u,v] = S[u%3,v%3], flat depth-1 build ----
            # [DVE] diag entries (+r): (0,0)(1,1)(2,2); (3,3)(4,4); (1,4)(4,1)
            nc.vector.tensor_tensor(
                _apx(SEXT, 0, (25, S), (6, 3)), _apx(P, 0, (36, S), (7, 3)),
                _apx(CST, 9, (12, S), (1, 3)), A.add)
            nc.vector.tensor_tensor(
                _apx(SEXT, 18, (25, S), (6, 2)), _apx(P, 0, (36, S), (7, 2)),
                _apx(CST, 9, (12, S), (1, 2)), A.add)
            nc.vector.tensor_tensor(
                _apx(SEXT, 9, (25, S), (12, 2)), _apx(P, 7, (36, S), (0, 2)),
                _apx(CST, 10, (12, S), (0, 2)), A.add)
            # [ACT] s01/s02 replicas (rows 0 and 3 merged in one op)
            nc.scalar.copy(  # (0,1)(0,2)(3,1)(3,2) <- P1,P2
                _apx(SEXT, 1, (25, S), (15, 2), (1, 2)),
                _apx(P, 1, (36, S), (0, 2), (1, 2)))
            nc.scalar.copy(  # (1,3)(3,4) <- P1
                _apx(SEXT, 8, (25, S), (11, 2)), _apx(P, 1, (36, S), (0, 2)))
            nc.scalar.copy(  # (4,3) <- P1
                _apx(SEXT, 23, (25, S), (1, 1)), _apx(P, 1, (36, S), (1, 1)))
            # [ACT] s02/s12 replicas
            nc.scalar.copy(  # (2,3) <- P2
                _apx(SEXT, 13, (25, S), (1, 1)), _apx(P, 2, (36, S), (1, 1)))
            nc.scalar.copy(  # (1,2)(2,1) <- P8
                _apx(SEXT, 7, (25, S), (4, 2)), _apx(P, 8, (36, S), (0, 2)))
            nc.scalar.copy(  # (2,4)(4,2) <- P8
                _apx(SEXT, 14, (25, S), (8, 2)), _apx(P, 8, (36, S), (0, 2)))

            nc.vector.tensor_tensor(  # right cols *= d (broadcast over rows)
                _apx(P, 3, (36, S), (6, 6), (1, 3)),
                _apx(P, 3, (36, S), (6, 6), (1, 3)),
                _apx(D3, 0, (3, S), (0, 6), (1, 3)), A.mult)
            nc.vector.tensor_tensor(  # diag += q
                _apx(P, 0, (36, S), (7, 6)), _apx(P, 0, (36, S), (7, 6)),
                _apx(CST, 3, (12, S), (1, 6)), A.add)

            # [DVE] vel' -= cf2 * t  (slots 0,1; waits on T3)
            nc.vector.scalar_tensor_tensor(
                _apx(X, 3, (6, S), (1, 2)), _apx(T3, 0, (3, S), (1, 2)),
                -sc["cf2"], _apx(X, 3, (6, S), (1, 2)), A.mult, A.add)

            # [Pool] maha side-chain part 1: YY = y (x) y
            _eng(nc).tensor_tensor(
                _apx(YY, 0, (9, S), (3, 3), (1, 3)),
                _apx(Y, 0, (3, S), (1, 3), (0, 3)),
                _apx(Y, 0, (3, S), (0, 3), (1, 3)), A.mult)


            # ---- cofactors C[a,b] = s[a+1,b+1]s[a+2,b+2]-s[a+1,b+2]s[a+2,b+1]
            nc.vector.tensor_tensor(
                _apx(PR1, 0, (9, S), (1, 9)), _apx(SEXT, 6, (25, S), (5, 3), (1, 3)),
                _apx(SEXT, 12, (25, S), (5, 3), (1, 3)), A.mult)
            nc.vector.tensor_tensor(
                _apx(PR2, 0, (9, S), (1, 9)), _apx(SEXT, 7, (25, S), (5, 3), (1, 3)),
                _apx(SEXT, 11, (25, S), (5, 3), (1, 3)), A.mult)
            nc.vector.tensor_tensor(
                _apx(C, 0, (9, S), (1, 9)), _apx(PR1, 0, (9, S), (1, 9)),
                _apx(PR2, 0, (9, S), (1, 9)), A.subtract)

            # ---- det, 1/det, ln(det) ----
            nc.vector.tensor_tensor(
                _apx(DP, 0, (3, S), (1, 3)), _apx(SEXT, 0, (25, S), (1, 3)),
                _apx(C, 0, (9, S), (1, 3)), A.mult)
            nc.vector.tensor_reduce(
                _apx(DET, 0, (1, S)), _apx(DP, 0, (3, S), (1, 3)),
                mybir.AxisListType.X, A.add)
            nc.vector.reciprocal(_apx(REC, 0, (1, S)), _apx(DET, 0, (1, S)))
            nc.scalar.activation(
                _apx(LD, 0, (1, S)), _apx(DET, 0, (1, S)), AF.Ln,
                accum_out=LDACC[:, t:t + 1])

            # ---- Sinv = C / det ----
            nc.vector.tensor_tensor(
                _apx(SINV, 0, (9, S), (1, 9)), _apx(C, 0, (9, S), (1, 9)),
                _apx(REC, 0, (1, S), (0, 9)), A.mult)

            # [Pool] maha contributions into MPACC
            _eng(nc).tensor_tensor(
                _apx(MPACC, 9 * S * t, (9, S), (1, 9)),
                _apx(SINV, 0, (9, S), (1, 9)),
                _apx(YY, 0, (9, S), (1, 9)), A.mult)

            # ---- K = Pc Sinv, stored transposed: K6[a,i] = K[i,a] ----
            for a in range(3):
                nc.vector.tensor_tensor(
                    _apx(KPR, 18 * a, (54, S), (3, 6), (1, 3)),
                    _apx(P, 0, (36, S), (6, 6), (1, 3)),
                    _apx(SINV, a, (9, S), (0, 6), (3, 3)), A.mult)
            nc.vector.tensor_reduce(  # (s,a) merged: one reduce for all K cols
                _apx(K6, 0, (6, 3 * S), (1, 6)),
                _apx(KPR, 0, (18, 3 * S), (3, 6), (1, 3)),
                mybir.AxisListType.X, A.add)

            # ---- x_new = x_pred + K y ----
            nc.vector.tensor_tensor(
                _apx(XP, 0, (18, S), (3, 6), (1, 3)),
                _apx(K6, 0, (18, S), (1, 6), (6, 3)),
                _apx(Y, 0, (3, S), (0, 6), (1, 3)), A.mult)
            nc.vector.tensor_reduce(
                _apx(XU, 0, (6, S), (1, 6)), _apx(XP, 0, (18, S), (3, 6), (1, 3)),
                mybir.AxisListType.X, A.add)
            nc.vector.tensor_tensor(
                _apx(X, 0, (6, S), (1, 6)), _apx(X, 0, (6, S), (1, 6)),
                _apx(XU, 0, (6, S), (1, 6)), A.add)

            # ---- P_new = P_pred - sum_a K[:,a] (x) Pc[:,a]^T ----
            # tmp_a[i,j] = K6[a,i] * P[j,a]; a=1,2 have slack -> Pool
            nc.vector.tensor_tensor(
                _apx(TMP0, 0, (36, S), (6, 6), (1, 6)),
                _apx(K6, 0, (18, S), (1, 6), (0, 6)),
                _apx(P, 0, (36, S), (0, 6), (6, 6)), A.mult)
            _eng(nc).tensor_tensor(
                _apx(TMP1, 0, (36, S), (6, 6), (1, 6)),
                _apx(K6, 6, (18, S), (1, 6), (0, 6)),
                _apx(P, 1, (36, S), (0, 6), (6, 6)), A.mult)
            _eng(nc).tensor_tensor(
                _apx(TMP2, 0, (36, S), (6, 6), (1, 6)),
                _apx(K6, 12, (18, S), (1, 6), (0, 6)),
                _apx(P, 2, (36, S), (0, 6), (6, 6)), A.mult)
            for TMP in (TMP0, TMP1, TMP2):
                nc.vector.tensor_tensor(
                    _apx(P, 0, (36, S), (1, 36)), _apx(P, 0, (36, S), (1, 36)),
                    _apx(TMP, 0, (36, S), (1, 36)), A.subtract)


def _derived_scalars(params, covariance_params):
    dyna = np.abs(params.astype(np.float64))
    fric, damp = float(dyna[0]), float(dyna[1])
    cp = covariance_params.astype(np.float64)
    a55 = 1.0 - DT * damp
    cK2 = DT * fric * G * K_SIGN
    sc = {
        "kdt2": 2.0 * K_SIGN / DT,
        "a55": a55,
        "cf2": DT * DT * fric * G,
        "cK2": cK2,
        "dcon": a55 - cK2,
        "p0v": 0.01 * DT * DT,
    }
    r_s = np.exp(cp[0:3])
    q_s = np.array([
        np.exp(cp[3]), np.exp(cp[3]), np.exp(cp[5]),
        np.exp(cp[4]) * DT * DT, np.exp(cp[4]) * DT * DT,
        np.exp(cp[6]) * DT * DT,
    ])
    return sc, r_s.astype(np.float32), q_s.astype(np.float32)


# ---------------------------------------------------------------------------
# Host-side runner (compiled callable cached across kernel() calls)
# ---------------------------------------------------------------------------

_CACHE = {}
LAST_RESULT = None


_ACT_PATCHED = False


def _patch_act_tables(bacc, mybir):
    """Make the act-table-load pass keep Exp and Ln only in the combined
    natural_log_exp_and_others set.  The default chooser picks the first
    set containing each function, which ping-pongs exp_and_others <->
    natural_log on every EKF step (~5.3us/step of table loads).  Set
    positions are preserved so act_func_set_id stays valid."""
    global _ACT_PATCHED
    if _ACT_PATCHED:
        return
    orig = bacc.get_activation_tables
    AF = mybir.ActivationFunctionType

    def patched(module_arch):
        tables = orig(module_arch)
        for name, funcs in tables.items():
            if name != "natural_log_exp_and_others":
                funcs.discard(AF.Exp)
                funcs.discard(AF.Ln)
        return tables

    bacc.get_activation_tables = patched
    _ACT_PATCHED = True


def _get_runner(key, sc, r_s, q_s, T=T_STEPS, reps=1):
    """Build the Bass kernel and a persistent jitted SPMD callable.

    run_bass_kernel_spmd re-creates its jax.jit wrapper on every call, which
    re-traces and hits compile caches each time; for a fast warm path we
    build the sharded executable once and reuse it.
    """
    key = (key, T, reps)
    if key in _CACHE:
        return _CACHE[key]

    import jax
    from jax.sharding import Mesh, PartitionSpec
    from jax.experimental.shard_map import shard_map
    import concourse.bacc as bacc
    import concourse.mybir as mybir
    import concourse.tile as tile
    from concourse import bass2jax

    f32 = mybir.dt.float32
    nc = bacc.Bacc("TRN2", target_bir_lowering=False, debug=False)
    meas_d = nc.dram_tensor("meas", [128, SLOTS * 3 * T_STEPS], f32,
                            kind="ExternalInput")
    x0_d = nc.dram_tensor("x0", [128, SLOTS * 6], f32, kind="ExternalInput")
    cst_d = nc.dram_tensor("cst", [128, SLOTS * 12], f32, kind="ExternalInput")
    out_d = nc.dram_tensor("out", [128, 1], f32, kind="ExternalOutput")
    with tile.TileContext(nc) as tc:
        _build_ekf(tc, out_d.ap(), meas_d.ap(), x0_d.ap(), cst_d.ap(), sc, T,
                   reps=reps)
    nc.compile()

    bass2jax.install_neuronx_cc_hook()
    out_aval = jax.core.ShapedArray((128, 1), np.float32)
    pid_name = (nc.partition_id_tensor.name if nc.partition_id_tensor
                else None)
    in_names = ("meas", "x0", "cst", "out") + (
        (pid_name,) if pid_name else ())

    def _body(meas, x0, cst, out_zero):
        operands = [meas, x0, cst, out_zero]
        if pid_name:
            operands.append(bass2jax.partition_id_tensor())
        outs = bass2jax._bass_exec_p.bind(
            *operands,
            out_avals=(out_aval,),
            in_names=in_names,
            out_names=("out",),
            lowering_input_output_aliases=(),
            sim_require_finite=True,
            sim_require_nnan=True,
            nc=nc,
        )
        return outs[0]

    devices = jax.devices()[:N_CORES]
    mesh = Mesh(np.asarray(devices), ("core",))
    sharded = jax.jit(
        shard_map(_body, mesh=mesh,
                  in_specs=(PartitionSpec("core"),) * 4,
                  out_specs=PartitionSpec("core"), check_rep=False),
        donate_argnums=(3,),
        keep_unused=True,
    )
    _CACHE[key] = (nc, sharded)
    return _CACHE[key]


def make_chain_runner(nc, k):
    """Jitted callable that executes the NEFF k times back-to-back inside
    one dispatch (serialized via the out-buffer dependency).  Used to
    measure pure device execution time without per-call RPC overhead."""
    import jax
    from jax.sharding import Mesh, PartitionSpec
    from jax.experimental.shard_map import shard_map
    from concourse import bass2jax

    out_aval = jax.core.ShapedArray((128, 1), np.float32)
    pid_name = (nc.partition_id_tensor.name if nc.partition_id_tensor
                else None)
    in_names = ("meas", "x0", "cst", "out") + (
        (pid_name,) if pid_name else ())

    def _body(meas, x0, cst, out_zero):
        out = out_zero
        for _ in range(k):
            operands = [meas, x0, cst, out]
            if pid_name:
                operands.append(bass2jax.partition_id_tensor())
            out = bass2jax._bass_exec_p.bind(
                *operands,
                out_avals=(out_aval,),
                in_names=in_names,
                out_names=("out",),
                lowering_input_output_aliases=(),
                sim_require_finite=True,
                sim_require_nnan=True,
                nc=nc,
            )[0]
        return out

    devices = jax.devices()[:N_CORES]
    mesh = Mesh(np.asarray(devices), ("core",))
    return jax.jit(
        shard_map(_body, mesh=mesh,
                  in_specs=(PartitionSpec("core"),) * 4,
                  out_specs=PartitionSpec("core"), check_rep=False),
        donate_argnums=(3,),
        keep_unused=True,
    )


def _prep_inputs_global(init_state, measurements, r_s, q_s):
    """Concatenated-over-cores inputs for the shard_map callable."""
    rq = r_s + q_s[0:3]
    cst_row = np.concatenate([r_s, q_s, rq]).astype(np.float32)      # 12
    cst = np.tile(cst_row, (N_CORES * 128, SLOTS))                   # [1024, 96]
    x_perm = init_state[:, [0, 1, 4, 2, 3, 5]].copy()
    x_perm[:, 3:6] *= DT
    meas_g = np.ascontiguousarray(measurements).reshape(
        N_CORES * 128, SLOTS * 3 * T_STEPS)
    x0_g = np.ascontiguousarray(x_perm).reshape(N_CORES * 128, SLOTS * 6)
    return meas_g, x0_g, cst


def kernel(params, covariance_params, init_state, measurements):
    global LAST_RESULT
    params = np.asarray(params, dtype=np.float32)
    covariance_params = np.asarray(covariance_params, dtype=np.float32)
    init_state = np.asarray(init_state, dtype=np.float32)
    measurements = np.asarray(measurements, dtype=np.float32)

    if (init_state.shape != (N_SEG, 6)
            or measurements.shape != (N_SEG, T_STEPS, 3)
            or params.shape != (4,) or covariance_params.shape != (7,)):
        return np.float32(_ekf_numpy(params, covariance_params, init_state,
                                     measurements) / init_state.shape[0])

    try:
        sc, r_s, q_s = _derived_scalars(params, covariance_params)
        key = (params.tobytes(), covariance_params.tobytes())
        nc, sharded = _get_runner(key, sc, r_s, q_s)
        meas_g, x0_g, cst_g = _prep_inputs_global(init_state, measurements,
                                                 r_s, q_s)
        out_zero = np.zeros((N_CORES * 128, 1), np.float32)
        out = np.asarray(sharded(meas_g, x0_g, cst_g, out_zero))
        LAST_RESULT = out
        total = np.sum(out.astype(np.float64))
        return np.float32(0.5 * total / N_SEG)
    except Exception:
        if os.environ.get("EKF_NO_FALLBACK"):
            raise
        return np.float32(_ekf_numpy(params, covariance_params, init_state,
                                     measurements) / init_state.shape[0])


# ---------------------------------------------------------------------------
# Pure-numpy fallback (reference-equivalent)
# ---------------------------------------------------------------------------


def _ekf_numpy(params, covariance_params, init_state, measurements):
    dyna = np.abs(params).astype(np.float32)
    fric, damp = dyna[0], dyna[1]
    cp = covariance_params
    R = np.diag(np.exp(cp[:3])).astype(np.float32)
    Q = np.diag(np.exp(np.stack(
        [cp[3], cp[3], cp[4], cp[4], cp[5], cp[6]]))).astype(np.float32)
    N = init_state.shape[0]
    midx = [0, 1, 4]
    x = init_state.copy()
    P = np.broadcast_to(np.eye(6, dtype=np.float32) * 0.01, (N, 6, 6)).copy()
    I6 = np.eye(6, dtype=np.float32)
    total = np.float64(0.0)
    for ti in range(measurements.shape[1]):
        z = measurements[:, ti, :]
        vel = x[:, 2:4]
        t = np.tanh(K_SIGN * vel)
        x_pred = np.concatenate(
            [x[:, 0:2] + DT * vel,
             vel - DT * (damp * vel + fric * G * t),
             x[:, 4:5] + DT * x[:, 5:6],
             x[:, 5:6] - DT * damp * x[:, 5:6]], axis=1).astype(np.float32)
        dv = 1.0 - DT * (damp + fric * G * K_SIGN * (1.0 - t * t))
        F = np.broadcast_to(I6, (N, 6, 6)).copy()
        F[:, 0, 2] = DT; F[:, 1, 3] = DT; F[:, 4, 5] = DT
        F[:, 2, 2] = dv[:, 0]; F[:, 3, 3] = dv[:, 1]
        F[:, 5, 5] = 1.0 - DT * damp
        P = np.einsum('nij,njk,nlk->nil', F, P, F) + Q
        y = z - x_pred[:, midx]
        ang = y[:, 2]
        ang = np.where(ang > 1.5 * np.pi, ang - TWO_PI,
                       np.where(ang < -1.5 * np.pi, ang + TWO_PI, ang))
        y[:, 2] = ang
        Sm = P[:, midx][:, :, midx] + R
        Sinv = np.linalg.inv(Sm.astype(np.float64)).astype(np.float32)
        K = np.einsum('nij,njk->nik', P[:, :, midx], Sinv)
        x = x_pred + np.einsum('nij,nj->ni', K, y)
        KH = np.zeros((N, 6, 6), np.float32)
        KH[:, :, midx] = K
        P = np.einsum('nij,njk->nik', I6 - KH, P)
        sign, logdet = np.linalg.slogdet(Sm.astype(np.float64))
        maha = np.einsum('ni,nij,nj->n', y, Sinv, y)
        total += 0.5 * np.sum(logdet + maha)
    return total



# revision 9
# speedup vs baseline: 1.0394x; 1.0394x over previous
"""Batched EKF negative-log-likelihood loss on 8 Trainium2 NeuronCores.

Data-parallel Bass/Tile kernel: the N=8192 segments are sharded 1024 per
core, laid out as 128 partitions x 8 segments.  The EKF state is kept in
permuted + DT-scaled coordinates [x, y, th, DT*vx, DT*vy, DT*om] so the
transition Jacobian becomes F = [[I, I], [0, D]] with D = diag(d0, d1, a55).

Compared to the straightforward full-6x6 formulation, this kernel:
  - stores only the unique blocks of the symmetric covariance: the left
    6x3 block [A; C], and the 3x3 velocity block V.  The state x rides as
    a 4th column of the left block and the innovation y as a 7th row, so
    the Kalman gain products and the covariance downdate fold the state
    update and the innovation in for free;
  - computes the innovation covariance S = A + C + C^T + V + R + Qa at
    the top of each step (it is independent of the tanh-based Jacobian),
    overlapping the 3x3 cofactor/determinant/inverse chain with the
    friction/velocity chain;
  - uses the Tanh activation directly (exp/tanh/square/copy live in one
    activation table set) and defers all logdet Ln evaluation to a single
    end-of-loop pass over the stored per-step determinants, so the scalar
    engine never swaps activation tables inside the loop;
  - spreads off-critical ops across the Pool (GPSIMD) engine.
"""

import os
import numpy as np

DT = 1.0 / 120.0
G = 9.81
K_SIGN = 100.0
TWO_PI = 2.0 * np.pi

N_CORES = 8
N_SEG = 8192
T_STEPS = 64
SEG_PER_CORE = N_SEG // N_CORES      # 1024
SLOTS = SEG_PER_CORE // 128          # 8 segments per partition

# Per-segment PX layout (36 floats):
#   EL[i][c] at 4*i+c, i in 0..5, c in 0..3:
#     cols 0-2 = [A (rows 0-2); C (rows 3-5)]  (left block of P)
#     col 3    = state x  (pos/th rows 0-2, DT-scaled vel rows 3-5)
#   y_neg[b] at 24+b  (innovation, negated: x_pred - z, theta-wrapped)
#   V[u][v]  at 27+3*u+v
OX = 3        # x column within EL
OY = 24       # y_neg offset
OV = 27       # V block offset

# ---------------------------------------------------------------------------
# Bass kernel builder
# ---------------------------------------------------------------------------


def _apx(tile_handle, off, *dims):
    """AP into `tile_handle` at element offset `off` with explicit
    (step, count) free dims, all 128 partitions."""
    from concourse.ap import AP

    base = tile_handle[:]
    ap = [list(base.ap[0])] + [[s, c] for (s, c) in dims]
    return AP(tensor=base.tensor, offset=base.offset + off, ap=ap)


def _build_ekf(tc, out_ap, meas_ap, px0_ap, cst_ap, sc, T, reps=1):
    import concourse.mybir as mybir

    nc = tc.nc
    f32 = mybir.dt.float32
    A = mybir.AluOpType
    S = SLOTS  # 8

    with tc.tile_pool(name="persist", bufs=1) as pp, \
         tc.tile_pool(name="temps", bufs=2) as tp:
        MEAS = pp.tile([128, S * 3 * T], f32, tag="MEAS")
        PX = pp.tile([128, S * 36], f32, tag="PX")
        T3 = pp.tile([128, S * 3], f32, tag="T3")
        D3 = pp.tile([128, S * 3], f32, tag="D3")
        CST = pp.tile([128, S * 14], f32, tag="CST")
        MPACC = pp.tile([128, 3 * S * T], f32, tag="MPACC")
        DETACC = pp.tile([128, S * T], f32, tag="DETACC")

        if meas_ap.shape[1] == S * 3 * T:
            nc.sync.dma_start(MEAS[:], meas_ap)
        else:  # timing variants with T < T_STEPS: strided slice of dram
            from concourse.ap import AP as _AP
            src = _AP(tensor=meas_ap.tensor, offset=meas_ap.offset,
                      ap=[list(meas_ap.ap[0]),
                          [meas_ap.shape[1] // S, S], [1, 3 * T]])
            nc.sync.dma_start(_apx(MEAS, 0, (3 * T, S), (1, 3 * T)), src)
        nc.sync.dma_start(CST[:], cst_ap)
        # T3 slot 2 stays 1.0 forever (makes d slot2 come out as a55)
        nc.vector.memset(_apx(T3, 2, (3, S), (1, 1)), 1.0)
        nc.vector.memset(_apx(D3, 2, (3, S), (1, 1)), sc["a55"])

        for _rep in range(reps):
            # PX0 image (P0 diag + x0) prepared host-side
            nc.sync.dma_start(PX[:], px0_ap)
            _ekf_steps(nc, tc, tp, sc, T, S, mybir, A,
                       MEAS, PX, T3, D3, CST, MPACC, DETACC)

        # ---- final: logdet via one Ln pass + maha reduce ----
        LD = tp.tile([128, S * T], f32, tag="LD")
        MT = tp.tile([128, 1], f32, tag="MT")
        LT = tp.tile([128, 1], f32, tag="LT")
        TOT = tp.tile([128, 1], f32, tag="TOT")
        AF = mybir.ActivationFunctionType
        nc.scalar.activation(LD[:], DETACC[:], AF.Ln, accum_out=LT[:])
        nc.vector.tensor_reduce(MT[:], MPACC[:], mybir.AxisListType.X, A.add)
        nc.vector.tensor_tensor(TOT[:], MT[:], LT[:], A.add)
        nc.sync.dma_start(out_ap, TOT[:])


def _ekf_steps(nc, tc, tp, sc, T, S, mybir, A,
               MEAS, PX, T3, D3, CST, MPACC, DETACC, capture=False):
    """One emission per timestep.  Engine split: DVE carries the covariance
    recursion and gain products; Pool (GPSIMD) takes the off-critical
    S-matrix replica/cofactor halves and the V-block downdate; ScalarE
    runs the tanh/Jacobian chain and the 5x5 replica copy."""
    f32 = mybir.dt.float32
    AF = mybir.ActivationFunctionType
    T3PI = 1.5 * np.pi
    all_dve = bool(os.environ.get("EKF_ALL_DVE"))
    pool = nc.vector if all_dve else nc.gpsimd

    for t in range(T):
        SEXT = tp.tile([128, S * 25], f32, tag="SEXT")
        PR1 = tp.tile([128, S * 9], f32, tag="PR1")
        PR2 = tp.tile([128, S * 9], f32, tag="PR2")
        COF = tp.tile([128, S * 9], f32, tag="COF")
        DP = tp.tile([128, S * 3], f32, tag="DP")
        REC = tp.tile([128, S], f32, tag="REC")
        TSQ = tp.tile([128, S * 2], f32, tag="TSQ")
        PRN = tp.tile([128, S * 63], f32, tag="PRN")
        NP = tp.tile([128, S * 21], f32, tag="NP")
        NS = tp.tile([128, S * 21], f32, tag="NS")
        TEL = tp.tile([128, S * 72], f32, tag="TEL")
        TV = tp.tile([128, S * 27], f32, tag="TV")

        # [ACT] t = tanh(K * v) on old velocities
        nc.scalar.activation(
            _apx(T3, 0, (3, S), (1, 2)), _apx(PX, 15, (36, S), (4, 2)),
            AF.Tanh, scale=sc["kdt"])

        # [DVE] predict, top half: [A | x_top] += [C | x_bot]
        nc.vector.tensor_tensor(
            _apx(PX, 0, (36, S), (4, 3), (1, 4)),
            _apx(PX, 0, (36, S), (4, 3), (1, 4)),
            _apx(PX, 12, (36, S), (4, 3), (1, 4)), A.add)
        # [DVE] A += C^T (transposed read of the C block)
        nc.vector.tensor_tensor(
            _apx(PX, 0, (36, S), (4, 3), (1, 3)),
            _apx(PX, 0, (36, S), (4, 3), (1, 3)),
            _apx(PX, 12, (36, S), (1, 3), (4, 3)), A.add)
        # [DVE] A += V
        nc.vector.tensor_tensor(
            _apx(PX, 0, (36, S), (4, 3), (1, 3)),
            _apx(PX, 0, (36, S), (4, 3), (1, 3)),
            _apx(PX, OV, (36, S), (3, 3), (1, 3)), A.add)

        # ---- SEXT[u,v] = S[u%3,v%3] with S = A' + diag(r + qa) ----
        # [ACT] rows 0-2, cols 0-2 = A'
        if all_dve:
            nc.vector.tensor_scalar_add(
                _apx(SEXT, 0, (25, S), (5, 3), (1, 3)),
                _apx(PX, 0, (36, S), (4, 3), (1, 3)), 0.0)
        else:
            nc.scalar.copy(
                _apx(SEXT, 0, (25, S), (5, 3), (1, 3)),
                _apx(PX, 0, (36, S), (4, 3), (1, 3)))
        # [Pool] rows 0-2, cols 3-4 = A' cols 0-1
        pool.tensor_scalar_add(
            _apx(SEXT, 3, (25, S), (5, 3), (1, 2)),
            _apx(PX, 0, (36, S), (4, 3), (1, 2)), 0.0)
        # [DVE] diag-class entries (u == v%3) += rq, via the rq5 pattern
        nc.vector.tensor_tensor(
            _apx(SEXT, 0, (25, S), (3, 5)),
            _apx(SEXT, 0, (25, S), (3, 5)),
            _apx(CST, 9, (14, S), (1, 5)), A.add)
        # [ACT] rows 3-4 = rows 0-1
        if all_dve:
            nc.vector.tensor_scalar_add(
                _apx(SEXT, 15, (25, S), (1, 10)),
                _apx(SEXT, 0, (25, S), (1, 10)), 0.0)
        else:
            nc.scalar.copy(
                _apx(SEXT, 15, (25, S), (1, 10)),
                _apx(SEXT, 0, (25, S), (1, 10)))

        # [DVE] innovation y_neg = x_pos - z, then theta wrap
        nc.vector.tensor_tensor(
            _apx(PX, OY, (36, S), (1, 3)),
            _apx(PX, OX, (36, S), (4, 3)),
            _apx(MEAS, 3 * t, (3 * T, S), (1, 3)), A.subtract)
        nc.vector.add_range_wrap(
            _apx(PX, OY + 2, (36, S), (1, 1)), _apx(PX, OY + 2, (36, S), (1, 1)),
            0.0, T3PI, TWO_PI)

        # [ACT] Jacobian diag: TSQ = t^2, D3 = cK2*t^2 + dcon
        if all_dve:
            nc.vector.tensor_tensor(
                _apx(TSQ, 0, (2, S), (1, 2)), _apx(T3, 0, (3, S), (1, 2)),
                _apx(T3, 0, (3, S), (1, 2)), A.mult)
            nc.vector.tensor_scalar(
                _apx(D3, 0, (3, S), (1, 2)), _apx(TSQ, 0, (2, S), (1, 2)),
                sc["cK2"], sc["dcon"], A.mult, A.add)
        else:
            nc.scalar.activation(
                _apx(TSQ, 0, (2, S), (1, 2)), _apx(T3, 0, (3, S), (1, 2)),
                AF.Square)
            nc.scalar.activation(
                _apx(D3, 0, (3, S), (1, 2)), _apx(TSQ, 0, (2, S), (1, 2)),
                AF.Copy, bias=sc["dcon"], scale=sc["cK2"])

        # [DVE] C += V (read before scaling), then C *= d rows
        nc.vector.tensor_tensor(
            _apx(PX, 12, (36, S), (4, 3), (1, 3)),
            _apx(PX, 12, (36, S), (4, 3), (1, 3)),
            _apx(PX, OV, (36, S), (3, 3), (1, 3)), A.add)
        nc.vector.tensor_tensor(
            _apx(PX, 12, (36, S), (4, 3), (1, 3)),
            _apx(PX, 12, (36, S), (4, 3), (1, 3)),
            _apx(D3, 0, (3, S), (1, 3), (0, 3)), A.mult)
        # [DVE] V = D V D + Qv  (two scales + diag add)
        nc.vector.tensor_tensor(
            _apx(PX, OV, (36, S), (3, 3), (1, 3)),
            _apx(PX, OV, (36, S), (3, 3), (1, 3)),
            _apx(D3, 0, (3, S), (1, 3), (0, 3)), A.mult)
        nc.vector.tensor_tensor(
            _apx(PX, OV, (36, S), (3, 3), (1, 3)),
            _apx(PX, OV, (36, S), (3, 3), (1, 3)),
            _apx(D3, 0, (3, S), (0, 3), (1, 3)), A.mult)
        # [Pool] V diag += qv ; A diag += qa
        pool.tensor_tensor(
            _apx(PX, OV, (36, S), (4, 3)), _apx(PX, OV, (36, S), (4, 3)),
            _apx(CST, 6, (14, S), (1, 3)), A.add)
        pool.tensor_tensor(
            _apx(PX, 0, (36, S), (5, 3)), _apx(PX, 0, (36, S), (5, 3)),
            _apx(CST, 3, (14, S), (1, 3)), A.add)

        # [ACT] vel *= a55;  [DVE] vel(x,y) -= cf2 * t
        if all_dve:
            nc.vector.tensor_scalar_mul(
                _apx(PX, 15, (36, S), (4, 3)), _apx(PX, 15, (36, S), (4, 3)),
                sc["a55"])
        else:
            nc.scalar.activation(
                _apx(PX, 15, (36, S), (4, 3)), _apx(PX, 15, (36, S), (4, 3)),
                AF.Copy, scale=sc["a55"])
        nc.vector.scalar_tensor_tensor(
            _apx(PX, 15, (36, S), (4, 2)), _apx(T3, 0, (3, S), (1, 2)),
            -sc["cf2"], _apx(PX, 15, (36, S), (4, 2)), A.mult, A.add)

        # ---- cofactors of S (via the 5x5 replica) ----
        nc.vector.tensor_tensor(
            _apx(PR1, 0, (9, S), (1, 9)), _apx(SEXT, 6, (25, S), (5, 3), (1, 3)),
            _apx(SEXT, 12, (25, S), (5, 3), (1, 3)), A.mult)
        pool.tensor_tensor(
            _apx(PR2, 0, (9, S), (1, 9)), _apx(SEXT, 7, (25, S), (5, 3), (1, 3)),
            _apx(SEXT, 11, (25, S), (5, 3), (1, 3)), A.mult)
        nc.vector.tensor_tensor(
            _apx(COF, 0, (9, S), (1, 9)), _apx(PR1, 0, (9, S), (1, 9)),
            _apx(PR2, 0, (9, S), (1, 9)), A.subtract)

        # ---- det into DETACC (logdet deferred), 1/det ----
        nc.vector.tensor_tensor(
            _apx(DP, 0, (3, S), (1, 3)), _apx(SEXT, 0, (25, S), (1, 3)),
            _apx(COF, 0, (9, S), (1, 3)), A.mult)
        nc.vector.tensor_reduce(
            _apx(DETACC, t * S, (1, S)), _apx(DP, 0, (3, S), (1, 3)),
            mybir.AxisListType.X, A.add)
        nc.vector.reciprocal(_apx(REC, 0, (1, S)), _apx(DETACC, t * S, (1, S)))

        # ---- N' = COF @ G,  G = [A'; C'; y_neg] read as columns of PX ----
        # PRN[a][j][b] = COF[a][b] * G[b][j];  N'[a][j] = sum_b PRN
        # (split by a: TENSOR3D allows only 3 free dims per AP)
        for a in range(3):
            nc.vector.tensor_tensor(
                _apx(PRN, 21 * a, (63, S), (3, 7), (1, 3)),
                _apx(COF, 3 * a, (9, S), (0, 7), (1, 3)),
                _apx(PX, 0, (36, S), (4, 7), (1, 3)), A.mult)
        nc.vector.tensor_reduce(
            _apx(NP, 0, (7, 3 * S), (1, 7)),
            _apx(PRN, 0, (21, 3 * S), (3, 7), (1, 3)),
            mybir.AxisListType.X, A.add)
        # [DVE] N = N' / det
        nc.vector.tensor_tensor(
            _apx(NS, 0, (21, S), (1, 21)), _apx(NP, 0, (21, S), (1, 21)),
            _apx(REC, 0, (1, S), (0, 21)), A.mult)

        # [Pool] maha contribution: y_neg . (Sinv y_neg) per segment
        pool.tensor_tensor(
            _apx(MPACC, 3 * S * t, (3, S), (1, 3)),
            _apx(PX, OY, (36, S), (1, 3)),
            _apx(NS, 6, (21, S), (7, 3)), A.mult)

        # ---- downdate: [ELx | V] -= Pc * N ----
        # TEL[a][i][c] = Pc[i][a]*N[a][c] for c in 0..2, and *N[a][6] at c=3
        for a in range(3):
            nc.vector.tensor_tensor(
                _apx(TEL, 24 * a, (72, S), (4, 6), (1, 3)),
                _apx(PX, a, (36, S), (4, 6), (0, 3)),
                _apx(NS, 7 * a, (21, S), (0, 6), (1, 3)), A.mult)
        nc.vector.tensor_tensor(
            _apx(TEL, 3, (72, S), (24, 3), (4, 6)),
            _apx(PX, 0, (36, S), (1, 3), (4, 6)),
            _apx(NS, 6, (21, S), (7, 3), (0, 6)), A.mult)
        # [Pool] TV[a][u][v] = C'[u][a]*N[a][3+v]
        for a in range(3):
            pool.tensor_tensor(
                _apx(TV, 9 * a, (27, S), (3, 3), (1, 3)),
                _apx(PX, 12 + a, (36, S), (4, 3), (0, 3)),
                _apx(NS, 7 * a + 3, (21, S), (0, 3), (1, 3)), A.mult)
        # [DVE] EL -= TEL_a  (state update rides in column 3)
        for a in range(3):
            nc.vector.tensor_tensor(
                _apx(PX, 0, (36, S), (1, 24)), _apx(PX, 0, (36, S), (1, 24)),
                _apx(TEL, 24 * a, (72, S), (1, 24)), A.subtract)
        # [Pool] V -= TV_a
        for a in range(3):
            pool.tensor_tensor(
                _apx(PX, OV, (36, S), (1, 9)), _apx(PX, OV, (36, S), (1, 9)),
                _apx(TV, 9 * a, (27, S), (1, 9)), A.subtract)
    if capture:
        return {"SEXT": SEXT, "COF": COF, "PRN": PRN, "NP": NP, "NS": NS,
                "TEL": TEL, "TV": TV, "REC": REC}


def _derived_scalars(params, covariance_params):
    dyna = np.abs(params.astype(np.float64))
    fric, damp = float(dyna[0]), float(dyna[1])
    cp = covariance_params.astype(np.float64)
    a55 = 1.0 - DT * damp
    cK2 = DT * fric * G * K_SIGN
    sc = {
        "kdt": K_SIGN / DT,
        "a55": a55,
        "cf2": DT * DT * fric * G,
        "cK2": cK2,
        "dcon": a55 - cK2,
        "p0v": 0.01 * DT * DT,
    }
    r_s = np.exp(cp[0:3])
    q_s = np.array([
        np.exp(cp[3]), np.exp(cp[3]), np.exp(cp[5]),
        np.exp(cp[4]) * DT * DT, np.exp(cp[4]) * DT * DT,
        np.exp(cp[6]) * DT * DT,
    ])
    return sc, r_s.astype(np.float32), q_s.astype(np.float32)


# ---------------------------------------------------------------------------
# Host-side runner (compiled callable cached across kernel() calls)
# ---------------------------------------------------------------------------

_CACHE = {}
LAST_RESULT = None


def _get_runner(key, sc, r_s, q_s, T=T_STEPS, reps=1):
    """Build the Bass kernel and a persistent jitted SPMD callable."""
    key = (key, T, reps)
    if key in _CACHE:
        return _CACHE[key]

    import jax
    from jax.sharding import Mesh, PartitionSpec
    from jax.experimental.shard_map import shard_map
    import concourse.bacc as bacc
    import concourse.mybir as mybir
    import concourse.tile as tile
    from concourse import bass2jax

    f32 = mybir.dt.float32
    nc = bacc.Bacc("TRN2", target_bir_lowering=False, debug=False)
    meas_d = nc.dram_tensor("meas", [128, SLOTS * 3 * T_STEPS], f32,
                            kind="ExternalInput")
    px0_d = nc.dram_tensor("px0", [128, SLOTS * 36], f32, kind="ExternalInput")
    cst_d = nc.dram_tensor("cst", [128, SLOTS * 14], f32, kind="ExternalInput")
    out_d = nc.dram_tensor("out", [128, 1], f32, kind="ExternalOutput")
    with tile.TileContext(nc) as tc:
        _build_ekf(tc, out_d.ap(), meas_d.ap(), px0_d.ap(), cst_d.ap(), sc, T,
                   reps=reps)
    nc.compile()

    bass2jax.install_neuronx_cc_hook()
    out_aval = jax.core.ShapedArray((128, 1), np.float32)
    pid_name = (nc.partition_id_tensor.name if nc.partition_id_tensor
                else None)
    in_names = ("meas", "px0", "cst", "out") + (
        (pid_name,) if pid_name else ())

    def _body(meas, px0, cst, out_zero):
        operands = [meas, px0, cst, out_zero]
        if pid_name:
            operands.append(bass2jax.partition_id_tensor())
        outs = bass2jax._bass_exec_p.bind(
            *operands,
            out_avals=(out_aval,),
            in_names=in_names,
            out_names=("out",),
            lowering_input_output_aliases=(),
            sim_require_finite=True,
            sim_require_nnan=True,
            nc=nc,
        )
        return outs[0]

    devices = jax.devices()[:N_CORES]
    mesh = Mesh(np.asarray(devices), ("core",))
    sharded = jax.jit(
        shard_map(_body, mesh=mesh,
                  in_specs=(PartitionSpec("core"),) * 4,
                  out_specs=PartitionSpec("core"), check_rep=False),
        donate_argnums=(3,),
        keep_unused=True,
    )
    _CACHE[key] = (nc, sharded)
    return _CACHE[key]


def make_chain_runner(nc, k):
    """Jitted callable that executes the NEFF k times back-to-back inside
    one dispatch (serialized via the out-buffer dependency)."""
    import jax
    from jax.sharding import Mesh, PartitionSpec
    from jax.experimental.shard_map import shard_map
    from concourse import bass2jax

    out_aval = jax.core.ShapedArray((128, 1), np.float32)
    pid_name = (nc.partition_id_tensor.name if nc.partition_id_tensor
                else None)
    in_names = ("meas", "px0", "cst", "out") + (
        (pid_name,) if pid_name else ())

    def _body(meas, px0, cst, out_zero):
        out = out_zero
        for _ in range(k):
            operands = [meas, px0, cst, out]
            if pid_name:
                operands.append(bass2jax.partition_id_tensor())
            out = bass2jax._bass_exec_p.bind(
                *operands,
                out_avals=(out_aval,),
                in_names=in_names,
                out_names=("out",),
                lowering_input_output_aliases=(),
                sim_require_finite=True,
                sim_require_nnan=True,
                nc=nc,
            )[0]
        return out

    devices = jax.devices()[:N_CORES]
    mesh = Mesh(np.asarray(devices), ("core",))
    return jax.jit(
        shard_map(_body, mesh=mesh,
                  in_specs=(PartitionSpec("core"),) * 4,
                  out_specs=PartitionSpec("core"), check_rep=False),
        donate_argnums=(3,),
        keep_unused=True,
    )


def _prep_inputs_global(init_state, measurements, r_s, q_s):
    """Concatenated-over-cores inputs for the shard_map callable."""
    rq = r_s + q_s[0:3]
    # rq5 covers SEXT diag-class offsets (0,3,6,9,12) =
    # positions (0,0),(0,3),(1,1),(1,4),(2,2) -> values rq[0,0,1,1,2]
    rq5 = rq[[0, 0, 1, 1, 2]]
    cst_row = np.concatenate([r_s, q_s, rq5]).astype(np.float32)     # 14
    cst = np.tile(cst_row, (N_CORES * 128, SLOTS))
    x_perm = init_state[:, [0, 1, 4, 2, 3, 5]].astype(np.float32).copy()
    x_perm[:, 3:6] *= DT
    meas_g = np.ascontiguousarray(measurements).reshape(
        N_CORES * 128, SLOTS * 3 * T_STEPS)
    # PX0 image: A diag 0.01 / V diag 0.01*DT^2 / x in EL column 3
    px_seg = np.zeros((N_SEG, 36), np.float32)
    px_seg[:, [0, 5, 10]] = 0.01
    px_seg[:, [OV, OV + 4, OV + 8]] = np.float32(0.01 * DT * DT)
    px_seg[:, [OX, OX + 4, OX + 8, OX + 12, OX + 16, OX + 20]] = x_perm
    px_g = px_seg.reshape(N_CORES * 128, SLOTS * 36)
    return meas_g, px_g, cst


def kernel(params, covariance_params, init_state, measurements):
    global LAST_RESULT
    params = np.asarray(params, dtype=np.float32)
    covariance_params = np.asarray(covariance_params, dtype=np.float32)
    init_state = np.asarray(init_state, dtype=np.float32)
    measurements = np.asarray(measurements, dtype=np.float32)

    if (init_state.shape != (N_SEG, 6)
            or measurements.shape != (N_SEG, T_STEPS, 3)
            or params.shape != (4,) or covariance_params.shape != (7,)):
        return np.float32(_ekf_numpy(params, covariance_params, init_state,
                                     measurements) / init_state.shape[0])

    try:
        sc, r_s, q_s = _derived_scalars(params, covariance_params)
        key = (params.tobytes(), covariance_params.tobytes())
        nc, sharded = _get_runner(key, sc, r_s, q_s)
        meas_g, px_g, cst_g = _prep_inputs_global(init_state, measurements,
                                                  r_s, q_s)
        out_zero = np.zeros((N_CORES * 128, 1), np.float32)
        out = np.asarray(sharded(meas_g, px_g, cst_g, out_zero))
        LAST_RESULT = out
        total = np.sum(out.astype(np.float64))
        return np.float32(0.5 * total / N_SEG)
    except Exception:
        if os.environ.get("EKF_NO_FALLBACK"):
            raise
        return np.float32(_ekf_numpy(params, covariance_params, init_state,
                                     measurements) / init_state.shape[0])


# ---------------------------------------------------------------------------
# Pure-numpy fallback (reference-equivalent)
# ---------------------------------------------------------------------------


def _ekf_numpy(params, covariance_params, init_state, measurements):
    dyna = np.abs(params).astype(np.float32)
    fric, damp = dyna[0], dyna[1]
    cp = covariance_params
    R = np.diag(np.exp(cp[:3])).astype(np.float32)
    Q = np.diag(np.exp(np.stack(
        [cp[3], cp[3], cp[4], cp[4], cp[5], cp[6]]))).astype(np.float32)
    N = init_state.shape[0]
    midx = [0, 1, 4]
    x = init_state.copy()
    P = np.broadcast_to(np.eye(6, dtype=np.float32) * 0.01, (N, 6, 6)).copy()
    I6 = np.eye(6, dtype=np.float32)
    total = np.float64(0.0)
    for ti in range(measurements.shape[1]):
        z = measurements[:, ti, :]
        vel = x[:, 2:4]
        t = np.tanh(K_SIGN * vel)
        x_pred = np.concatenate(
            [x[:, 0:2] + DT * vel,
             vel - DT * (damp * vel + fric * G * t),
             x[:, 4:5] + DT * x[:, 5:6],
             x[:, 5:6] - DT * damp * x[:, 5:6]], axis=1).astype(np.float32)
        dv = 1.0 - DT * (damp + fric * G * K_SIGN * (1.0 - t * t))
        F = np.broadcast_to(I6, (N, 6, 6)).copy()
        F[:, 0, 2] = DT; F[:, 1, 3] = DT; F[:, 4, 5] = DT
        F[:, 2, 2] = dv[:, 0]; F[:, 3, 3] = dv[:, 1]
        F[:, 5, 5] = 1.0 - DT * damp
        P = np.einsum('nij,njk,nlk->nil', F, P, F) + Q
        y = z - x_pred[:, midx]
        ang = y[:, 2]
        ang = np.where(ang > 1.5 * np.pi, ang - TWO_PI,
                       np.where(ang < -1.5 * np.pi, ang + TWO_PI, ang))
        y[:, 2] = ang
        Sm = P[:, midx][:, :, midx] + R
        Sinv = np.linalg.inv(Sm.astype(np.float64)).astype(np.float32)
        K = np.einsum('nij,njk->nik', P[:, :, midx], Sinv)
        x = x_pred + np.einsum('nij,nj->ni', K, y)
        KH = np.zeros((N, 6, 6), np.float32)
        KH[:, :, midx] = K
        P = np.einsum('nij,njk->nik', I6 - KH, P)
        sign, logdet = np.linalg.slogdet(Sm.astype(np.float64))
        maha = np.einsum('ni,nij,nj->n', y, Sinv, y)
        total += 0.5 * np.sum(logdet + maha)
    return total
